# revision 53
# baseline (speedup 1.0000x reference)
"""Trainium2 Bass kernel for the CIFAR10 Monarch MLP (7 monarch layers + log_softmax).

Strategy
--------
Pure data parallel over 8 NeuronCores: each core takes a 1024-row batch shard;
the ~9M-param block-diagonal weights are replicated.

On-device dataflow is feature-major: activations live in SBUF as
[features (128-partition tiles), batch (free dim)], fully SBUF-resident across
all layers; only x, the weights and the final log-probs cross HBM.

Layers 1-4 keep the monarch two-GEMM structure expressed as block-sparse
matmuls over the *effective* weight matrices (butterfly permutation folded
into W1 on the host).  The mid layouts are chosen to minimise 128x128 tile
count: layers 1-2 group mid features by output block l with k-aligned
sub-groups; layers 3-4 use k-pure mid tiles (every mid tile draws from one
input block, with 2 or 4 l-cells packed per tile and the unused rows made
zero by construction), which cuts L3 from 80 to 64 and L4 from 32 to 24
tiles.  Layers 5-6 are fused into a single dense GEMM each.  Layer 7 is
fused and computed batch-major (activation tile as the stationary operand),
so the logits land as [batch, 10] in PSUM and log_softmax needs no
transposes.

GEMM1 of layers 1-2 runs in fp8-e4m3 DoubleRow mode: two 128-row
contraction tiles per instruction at 2x PE rate (157 TF/s).  Each W1 is
pre-scaled by a power of two to rms~2 for e4m3 and the inverse scale is
folded into that layer's bf16 W2, so no extra runtime ops appear; layer 1
reads fp8 x straight from HBM and layer 1's G2 evicts PSUM->fp8 (padded to
full 128-row tiles so DoubleRow contracts whole pairs).  Layer 1's GEMM2
additionally contracts the first two mid tiles of each l-chain as one fp8
DoubleRow pair accumulated into the same PSUM as the remaining bf16
blocks: those mid tiles are evicted as fp8 at rms~0.19 and the fp8 W2 part
carries the inverse scale, so the scale product is exactly 1 and no fixup
op is needed.  Full fp8 on any further GEMM was measured (host-sim + HW)
to push max rel err past the 2e-2 gate; this config lands at 1.59e-2 vs
2.3e-3 all-bf16.

The tensor engine needs ~3us of continuous work to reach max clock, so the
startup burns the DMA wait with dummy 256-row matmuls; a mid-stream PE gap
resets the ramp, so the warm-up is sized to hand over to real work just as
the first x/W1 slices land.  The tail pipelines dense layers / form-B /
softmax over the two 512-column batch chunks so the serial DVE softmax
chain overlaps PE work, and y is written in on-chip [partition, chunk,
class] order (the host unpermutes) to avoid a 1024-descriptor scatter DMA.

log_softmax avoids the Ln activation function entirely: ln(S) is computed
with a DVE exponent-extraction estimate refined by one Newton step that only
needs Exp.  Every activation function used (Copy / Relu / Exp) lives in one
activation-table set, so the program loads act tables exactly once - extra
InstLoadActFuncSet switches measurably slow *all* PE matmuls by ~20%.
"""

import os as _os

import numpy as np

import concourse.bacc as bacc_mod
import concourse.mybir as mybir
import concourse.tile as tile
from concourse.bass_utils import run_bass_kernel_spmd

# ----------------------------------------------------------------- problem dims
BATCH = 8192
IN_FEATURES = 3072
NCORES = 8
BPC = BATCH // NCORES          # 1024 batch rows per core
NOUT = 10

SHAPES = [((4, 750, 768), (4, 750, 750)),
          ((4, 500, 750), (4, 500, 500)),
          ((4, 250, 500), (4, 250, 250)),
          ((4, 125, 250), (4, 125, 125)),
          ((4, 50, 125), (4, 50, 50)),
          ((4, 25, 50), (4, 25, 25)),
          ((4, 3, 25), (4, 3, 3))]
NLAYERS = 7
NMONARCH = 4                   # layers 0..3 stay factored; 4..5 dense; 6 form-B

F32 = mybir.dt.float32
I32 = mybir.dt.int32
FP8 = mybir.dt.float8e4
ACT_DT_NAME = _os.environ.get("KERNEL_MM_DT", "bf16")
ACT_DT = {"fp32": mybir.dt.float32,
          "fp32r": mybir.dt.float32r,
          "bf16": mybir.dt.bfloat16}[ACT_DT_NAME]
# GEMM1 of layers 1-2 runs in fp8-e4m3 DoubleRow mode (2 contraction tiles
# per instruction at 2x rate).  Each W1 is pre-scaled to unit-ish rms for
# e4m3 and the inverse scale is folded into that layer's (bf16) W2, so no
# extra runtime ops appear.  Layer 1's activations enter fp8 from HBM;
# layer 2's are evicted PSUM->fp8 by layer 1's G2 (padded to full 128-row
# tiles so DoubleRow can contract whole pairs).
L1G1_FP8 = _os.environ.get("KERNEL_L1G1_FP8", "1") == "1"
FP8_G1_LAYERS = (0, 1) if L1G1_FP8 else ()
# Partial fp8 on L1-GEMM2: the first 2*NP8 mid tiles of each l-chain are
# evicted as fp8 (scaled to rms ~0.19) and contracted as DoubleRow pairs
# into the same PSUM accumulation as the remaining bf16 blocks.  The scale
# product of the fp8 W2 part and mid8 is 1, so no fixup op is needed.
L1G2_NP8 = int(_os.environ.get("KERNEL_L1G2_NP8", "1")) if L1G1_FP8 else 0
S1_FIXED = 64.0          # W1 fp8 scale for layers 1-2 (randn/sqrt(750ish))
MID8_SCALE = 0.1875      # mid8 = mid_true * MID8_SCALE (rms ~0.19)

LN2_OVER_2P23 = float(np.log(2.0) / (1 << 23))   # 8.262958e-08
EXP_BITS_BIAS = 1065353216.0                     # bitcast(1.0f)


# ------------------------------------------------------------------ layouts
class Layout:
    """Placement of 4 feature blocks of size Sb into 128-partition tiles."""

    @classmethod
    def from_positions(cls, Sb, ntiles, feat_tile, feat_row):
        self = object.__new__(cls)
        self.Sb = Sb
        self.ntiles = ntiles
        self.feat_tile = feat_tile
        self.feat_row = feat_row
        self._finish()
        return self

    def _finish(self):
        self.valid = np.zeros(self.ntiles, np.int64)
        for k in range(4):
            for t, r in zip(self.feat_tile[k], self.feat_row[k]):
                self.valid[t] = max(self.valid[t], r + 1)
        self.grow = [self.feat_tile[k] * 128 + self.feat_row[k]
                     for k in range(4)]
        self.tiles_of_block = [sorted(set(self.feat_tile[k].tolist()))
                               for k in range(4)]


def simple_layout(Sb):
    """Blocks >= 128 rows: chunked over dedicated tiles. 65..127: one tile
    each. <= 64: packed at 32-aligned offsets."""
    if Sb >= 128:
        cpb = (Sb + 127) // 128
        ft, fr = [], []
        for k in range(4):
            i = np.arange(Sb)
            ft.append(k * cpb + i // 128)
            fr.append(i % 128)
        return Layout.from_positions(Sb, 4 * cpb, ft, fr)
    stride = ((Sb + 31) // 32) * 32
    bpt = max(1, 128 // stride)
    ntiles = (4 + bpt - 1) // bpt
    ft, fr = [], []
    for k in range(4):
        i = np.arange(Sb)
        ft.append(np.full(Sb, k // bpt, np.int64))
        fr.append((k % bpt) * stride + i)
    return Layout.from_positions(Sb, ntiles, ft, fr)


def grouped_mid_layout(R, Q):
    """Mid layout with features regrouped by input block k (R >= 125)."""
    cpb = max(1, (R + 127) // 128)
    block_rows = cpb * 128
    Gp = block_rows // 4
    ft, fr = [], []
    for l in range(4):
        rs = np.arange(R)
        ks = (4 * rs + l) // Q
        pos = np.empty(R, np.int64)
        for k in range(4):
            idx = rs[ks == k]
            assert len(idx) <= Gp
            pos[idx] = k * Gp + np.arange(len(idx))
        ft.append(l * cpb + pos // 128)
        fr.append(pos % 128)
    return Layout.from_positions(R, 4 * cpb, ft, fr)


def kpure_mid_layout(R, Q, slot):
    """k-pure mid tiles: every tile draws from a single input block k, so a
    G1 chain needs only that k's input tiles.  slot=64: two l's per tile
    (tile k*2 + l//2, cells at 64-row offsets; needs cells <= 64).  slot=32:
    one tile per k holding all four l cells at 32-row offsets (cells <= 32)."""
    ft, fr = [], []
    ntiles = 8 if slot == 64 else 4
    for l in range(4):
        rs = np.arange(R)
        ks = (4 * rs + l) // Q
        pos_t = np.empty(R, np.int64)
        pos_r = np.empty(R, np.int64)
        for k in range(4):
            idx = rs[ks == k]
            assert len(idx) <= slot
            if slot == 64:
                pos_t[idx] = k * 2 + l // 2
                pos_r[idx] = (l % 2) * 64 + np.arange(len(idx))
            else:
                pos_t[idx] = k
                pos_r[idx] = l * 32 + np.arange(len(idx))
        ft.append(pos_t)
        fr.append(pos_r)
    return Layout.from_positions(R, ntiles, ft, fr)


def build_full_mats(w1_shape, w2_shape, lin, lmid, lout):
    """Shapes of the dense effective matrices (values filled on the host)."""
    return ((lin.ntiles * 128, lmid.ntiles * 128),
            (lmid.ntiles * 128, lout.ntiles * 128))


class LayerPlan:
    """Factored (monarch) layer: two block-sparse GEMMs."""

    def __init__(self, li, w1_shape, w2_shape, in_layout):
        _, Q, P = w1_shape
        _, S, R = w2_shape
        self.li, self.P, self.Q, self.R, self.S = li, P, Q, R, S
        self.lin = in_layout
        cell_max = max(
            int(np.sum((4 * np.arange(R) + l) // Q == k))
            for l in range(4) for k in range(4))
        if cell_max <= 32:
            self.lmid = kpure_mid_layout(R, Q, 32)
        elif cell_max <= 64:
            self.lmid = kpure_mid_layout(R, Q, 64)
        elif R >= 125:
            self.lmid = grouped_mid_layout(R, Q)
        else:
            self.lmid = simple_layout(R)
        self.shared_mid = cell_max <= 64
        self.lout = simple_layout(S)
        self.ngroups = 2 if li == 0 else 1
        self._build()

    def _build(self):
        Q, R, S = self.Q, self.R, self.S
        ks_of = [(4 * np.arange(R) + l) // Q for l in range(4)]

        need1 = {}
        for l in range(4):
            for r in range(R):
                mt = int(self.lmid.feat_tile[l][r])
                k = int(ks_of[l][r])
                need1.setdefault(mt, set()).update(self.lin.tiles_of_block[k])
        self.g1_chains = {mt: sorted(its) for mt, its in need1.items()}
        self.w1_blocks = [(mt, it) for mt in sorted(need1)
                          for it in self.g1_chains[mt]]
        self.w1_block_of = {p: i for i, p in enumerate(self.w1_blocks)}

        need2 = {}
        for l in range(4):
            for s in range(S):
                ot = int(self.lout.feat_tile[l][s])
                need2.setdefault(ot, set()).update(self.lmid.tiles_of_block[l])
        self.g2_chains = {ot: sorted(mts) for ot, mts in need2.items()}
        self.w2_blocks = [(ot, mt) for ot in sorted(need2)
                          for mt in self.g2_chains[ot]]
        self.w2_block_of = {p: i for i, p in enumerate(self.w2_blocks)}

        self.mid_tiles_of_l = [self.lmid.tiles_of_block[l] for l in range(4)]
        self.out_tiles_of_l = [self.lout.tiles_of_block[l] for l in range(4)]

    def group_lset(self, g):
        return range(4) if self.ngroups == 1 else range(2 * g, 2 * g + 2)

    def group_w1range(self, g):
        mts = {t for l in self.group_lset(g) for t in self.mid_tiles_of_l[l]}
        idxs = [i for i, (mt, _) in enumerate(self.w1_blocks) if mt in mts]
        assert idxs == list(range(idxs[0], idxs[0] + len(idxs)))
        return idxs[0], len(idxs)

    def group_w2range(self, g):
        ots = {t for l in self.group_lset(g) for t in self.out_tiles_of_l[l]}
        idxs = [i for i, (ot, _) in enumerate(self.w2_blocks) if ot in ots]
        assert idxs == list(range(idxs[0], idxs[0] + len(idxs)))
        return idxs[0], len(idxs)

    def full_mats(self, w1, w2):
        """Dense effective matrices (butterfly folded into W1)."""
        Q, R = self.Q, self.R
        W1full = np.zeros((self.lin.ntiles * 128, self.lmid.ntiles * 128),
                          np.float32)
        W2full = np.zeros((self.lmid.ntiles * 128, self.lout.ntiles * 128),
                          np.float32)
        for l in range(4):
            js = 4 * np.arange(R) + l
            ks, qs = js // Q, js % Q
            mcols = self.lmid.grow[l]
            for k in range(4):
                sel = np.where(ks == k)[0]
                if len(sel) == 0:
                    continue
                W1full[np.ix_(self.lin.grow[k], mcols[sel])] = \
                    np.ascontiguousarray(w1[k, qs[sel], :].T)
            W2full[np.ix_(self.lmid.grow[l], self.lout.grow[l])] = \
                np.ascontiguousarray(w2[l].T)
        return W1full, W2full

    @property
    def g1_fp8(self):
        return self.li in FP8_G1_LAYERS

    def build_weights(self, w1, w2):
        """Host: gather the nonzero 128x128 tiles into [128, nblocks*128].
        For the fp8 layer, W1 is scaled by a power of two to rms~2 (e4m3
        sweet spot), quantized, and packed as DoubleRow pairs
        [128, npairs, 2, 128]; W2 absorbs the inverse scale."""
        W1full, W2full = self.full_mats(w1, w2)
        W1m = np.zeros((128, 128 * len(self.w1_blocks)), np.float32)
        for i, (mt, it) in enumerate(self.w1_blocks):
            W1m[:, i * 128:(i + 1) * 128] = \
                W1full[it * 128:(it + 1) * 128, mt * 128:(mt + 1) * 128]
        W2m = np.zeros((128, 128 * len(self.w2_blocks)), np.float32)
        for i, (ot, mt) in enumerate(self.w2_blocks):
            W2m[:, i * 128:(i + 1) * 128] = \
                W2full[mt * 128:(mt + 1) * 128, ot * 128:(ot + 1) * 128]
        if self.g1_fp8:
            s = S1_FIXED
            W1m = (W1m * s).reshape(128, len(self.w1_blocks) // 2, 2, 128)
            W2m = W2m * (1.0 / s)
        if self.li == 0 and L1G2_NP8 > 0:
            # split W2 per out tile: first 2*NP8 chain blocks become fp8
            # DoubleRow pairs (scaled 1/MID8_SCALE to pair with mid8 so the
            # product lands at natural scale in the shared PSUM), rest bf16.
            # W2m rows here already carry the 1/s fold; undo it for the fp8
            # part since mid8 is scaled from mid_true, not s*mid_true.
            n8 = 2 * L1G2_NP8
            not_ = len(self.g2_chains)
            W2m8 = np.zeros((128, not_, L1G2_NP8, 2, 128), np.float32)
            W2mB = np.zeros((128, not_ * (6 - n8) * 128), np.float32)
            for i, (ot, mt) in enumerate(self.w2_blocks):
                j = i % 6
                blk = W2m[:, i * 128:(i + 1) * 128]
                if j < n8:
                    W2m8[:, ot, j // 2, j % 2, :] = \
                        blk * (S1_FIXED / MID8_SCALE)
                else:
                    bi = ot * (6 - n8) + (j - n8)
                    W2mB[:, bi * 128:(bi + 1) * 128] = blk
            return W1m, (W2m8, W2mB)
        return W1m, W2m


class DensePlan:
    """Fused layer: one dense GEMM over the product W1eff @ W2eff."""

    def __init__(self, li, w1_shape, w2_shape, in_layout):
        self.li = li
        self.fact = LayerPlan(li, w1_shape, w2_shape, in_layout)
        self.lin = in_layout
        self.lout = self.fact.lout
        self.blocks = [(ot, it)
                       for ot in range(self.lout.ntiles)
                       for it in range(self.lin.ntiles)]
        self.block_of = {p: i for i, p in enumerate(self.blocks)}

    def build_weights(self, w1, w2):
        W1full, W2full = self.fact.full_mats(w1, w2)
        Wd = W1full @ W2full
        Wm = np.zeros((128, 128 * len(self.blocks)), np.float32)
        for i, (ot, it) in enumerate(self.blocks):
            Wm[:, i * 128:(i + 1) * 128] = \
                Wd[it * 128:(it + 1) * 128, ot * 128:(ot + 1) * 128]
        return Wm


class FormBPlan:
    """Final layer: fused dense [in_rows x NOUT], computed batch-major with
    the activation tile as the stationary operand."""

    def __init__(self, li, w1_shape, w2_shape, in_layout):
        self.li = li
        self.fact = LayerPlan(li, w1_shape, w2_shape, in_layout)
        self.lin = in_layout
        assert self.lin.ntiles == 1
        self.in_valid = int(self.lin.valid[0])

    def build_weights(self, w1, w2):
        W1full, W2full = self.fact.full_mats(w1, w2)
        Wd = W1full @ W2full                       # [in_rows, out_grow cols]
        lout = self.fact.lout
        cols = [int(lout.grow[n // 3][n % 3]) for n in range(NOUT)]
        W = np.zeros((128, NOUT), np.float32)
        W[:self.in_valid + 0, :] = Wd[:self.in_valid, cols][: 128]
        return W


def build_plans():
    plans = []
    lin = simple_layout(SHAPES[0][0][2])
    for i, (s1, s2) in enumerate(SHAPES):
        if i < NMONARCH:
            pl = LayerPlan(i, s1, s2, lin)
        elif i < NLAYERS - 1:
            pl = DensePlan(i, s1, s2, lin)
        else:
            pl = FormBPlan(i, s1, s2, lin)
        plans.append(pl)
        lin = pl.lout if i < NLAYERS - 1 else None
    return plans


# --------------------------------------------------- numpy model of the schedule
def numpy_forward(plans, weights, xT):
    B = xT.shape[1]
    h = np.zeros((plans[0].lin.ntiles * 128, B), np.float32)
    h[:xT.shape[0]] = xT
    for pl in plans:
        if isinstance(pl, LayerPlan):
            W1m, W2m = weights[pl.li]
            mid = np.zeros((pl.lmid.ntiles * 128, B), np.float32)
            for mt, its in pl.g1_chains.items():
                V = pl.lmid.valid[mt]
                acc = np.zeros((V, B), np.float32)
                for it in its:
                    ln = pl.lin.valid[it]
                    b = pl.w1_block_of[(mt, it)]
                    acc += W1m[0:ln, b * 128:b * 128 + V].T @ \
                        h[it * 128: it * 128 + ln]
                mid[mt * 128: mt * 128 + V] = acc
            out = np.zeros((pl.lout.ntiles * 128, B), np.float32)
            for ot, mts in pl.g2_chains.items():
                V = pl.lout.valid[ot]
                acc = np.zeros((V, B), np.float32)
                for mt in mts:
                    ln = pl.lmid.valid[mt]
                    b = pl.w2_block_of[(ot, mt)]
                    acc += W2m[0:ln, b * 128:b * 128 + V].T @ \
                        mid[mt * 128: mt * 128 + ln]
                out[ot * 128: ot * 128 + V] = acc
            h = np.maximum(out, 0.0)
        elif isinstance(pl, DensePlan):
            Wm = weights[pl.li]
            out = np.zeros((pl.lout.ntiles * 128, B), np.float32)
            for ot in range(pl.lout.ntiles):
                V = pl.lout.valid[ot]
                acc = np.zeros((V, B), np.float32)
                for it in range(pl.lin.ntiles):
                    ln = pl.lin.valid[it]
                    b = pl.block_of[(ot, it)]
                    acc += Wm[0:ln, b * 128:b * 128 + V].T @ \
                        h[it * 128: it * 128 + ln]
                out[ot * 128: ot * 128 + V] = acc
            h = np.maximum(out, 0.0)
        else:
            W = weights[pl.li]                     # [128, NOUT]
            ln = pl.in_valid
            logits = h[0:ln, :].T @ W[0:ln, :]     # [B, NOUT]
            t = logits
            s = np.exp(t).sum(axis=1, keepdims=True)
            return t - np.log(s)
    raise AssertionError


# ------------------------------------------------------------------ bass program
def build_program(plans):
    nc = bacc_mod.Bacc()

    x_dt = FP8 if L1G1_FP8 else ACT_DT
    # partition-major x in HBM: each DMA slice is one contiguous run per
    # partition (vs one run per tile), cutting startup descriptor count
    xT = nc.dram_tensor("xT", [128, plans[0].lin.ntiles, BPC], x_dt,
                        kind="ExternalInput")
    wdram = []
    for i, p in enumerate(plans):
        if isinstance(p, LayerPlan):
            if p.g1_fp8:
                w1t = nc.dram_tensor(
                    f"w1c_{i}", [128, len(p.w1_blocks) // 2, 2, 128],
                    FP8, kind="ExternalInput")
            else:
                w1t = nc.dram_tensor(
                    f"w1c_{i}", [128, 128 * len(p.w1_blocks)],
                    ACT_DT, kind="ExternalInput")
            if i == 0 and L1G2_NP8 > 0:
                n8 = 2 * L1G2_NP8
                w2t = (
                    nc.dram_tensor(f"w2c8_{i}",
                                   [128, len(p.g2_chains), L1G2_NP8, 2, 128],
                                   FP8, kind="ExternalInput"),
                    nc.dram_tensor(f"w2c_{i}",
                                   [128, len(p.g2_chains) * (6 - n8) * 128],
                                   ACT_DT, kind="ExternalInput"))
            else:
                w2t = nc.dram_tensor(f"w2c_{i}",
                                     [128, 128 * len(p.w2_blocks)],
                                     ACT_DT, kind="ExternalInput")
            wdram.append((w1t, w2t))
        elif isinstance(p, DensePlan):
            wdram.append(nc.dram_tensor(f"wdc_{i}", [128, 128 * len(p.blocks)],
                                        ACT_DT, kind="ExternalInput"))
        else:
            wdram.append(nc.dram_tensor(f"w7c_{i}", [128, NOUT], ACT_DT,
                                        kind="ExternalInput"))
    # y stays in the on-chip [partition, chunk, class] order; the host
    # unpermutes (batch row = chunk*128 + partition).  A [BPC, NOUT] dram
    # layout costs ~8us at kernel end: 1024 scattered 40-byte descriptors.
    y = nc.dram_tensor("y", [128, BPC // 128, NOUT], F32, kind="ExternalOutput")

    with tile.TileContext(nc) as tc:
        with (
            tc.tile_pool(name="sb", bufs=1) as sb,
            tc.tile_pool(name="ps", bufs=1, space="PSUM") as ps,
        ):
            evict_flip = [0]

            def evict(dst_ap, src_ap, relu, scale=None):
                e = evict_flip[0] = evict_flip[0] ^ 1
                if scale is not None:
                    if e:
                        nc.vector.tensor_scalar_mul(dst_ap, src_ap, scale)
                    else:
                        nc.scalar.activation(
                            dst_ap, src_ap,
                            mybir.ActivationFunctionType.Copy, scale=scale)
                elif relu:
                    if e:
                        nc.vector.tensor_scalar_max(dst_ap, src_ap, 0.0)
                    else:
                        nc.scalar.activation(dst_ap, src_ap,
                                             mybir.ActivationFunctionType.Relu)
                else:
                    if e:
                        nc.vector.tensor_copy(dst_ap, src_ap)
                    else:
                        nc.scalar.copy(dst_ap, src_ap)

            # ---- PE p-state warm-up: the tensor engine needs ~3us of
            # continuous work to reach max clock, and the first real chains
            # otherwise run 2x slow while DMA still streams x/weights.  Burn
            # the idle startup window with dummy matmuls on a zeroed scratch
            # tile (results discarded).  memset on gpsimd (idle and ready
            # ~1.5us before the vector engine at program start) and 256-row
            # warm-ups so the ramp completes with minimal overshoot.
            scr = sb.tile([128, 256], ACT_DT, name="scr", tag="scr")
            nc.gpsimd.memset(scr[:, :], 0.0)
            pwarm = ps.tile([128, 256], F32, name="pwarm", tag="p7b", bufs=2)
            for _ in range(16):
                nc.tensor.matmul(pwarm[:, :], scr[0:128, 0:128], scr[:, 0:256],
                                 start=True, stop=True)

            # ---- startup: first weight slices before/interleaved with x ----
            pl0 = plans[0]
            b1_0, b1_n = pl0.group_w1range(0)
            b2_0, b2_n = pl0.group_w2range(0)
            if pl0.g1_fp8:
                w1sb0 = sb.tile([128, b1_n // 2, 2, 128], FP8,
                                name="w1sb_0_0", tag="w1")
            else:
                w1sb0 = sb.tile([128, b1_n * 128], ACT_DT, name="w1sb_0_0",
                                tag="w1")
            NP8_2 = 2 * L1G2_NP8
            if L1G2_NP8 > 0:
                w2sb0 = (
                    sb.tile([128, 12, L1G2_NP8, 2, 128], FP8,
                            name="w2sb8_0_0", tag="w28"),
                    sb.tile([128, 12 * (6 - NP8_2) * 128], ACT_DT,
                            name="w2sb_0_0", tag="w2"))
            else:
                w2sb0 = sb.tile([128, b2_n * 128], ACT_DT, name="w2sb_0_0",
                                tag="w2")
            hin = sb.tile([128, pl0.lin.ntiles, BPC], x_dt,
                          name="h_in0", tag="hA")

            w1d0, w2d0 = wdram[0]
            # G1 chains of group 0 are emitted interleaved across l=0,1 (see
            # below); ship weight slices in that order, interleaved with x.
            g0_mts = []
            for a, b in zip(pl0.mid_tiles_of_l[0], pl0.mid_tiles_of_l[1]):
                g0_mts.extend((a, b))
            w1_order = []        # (block_start, block_count) per chain
            for mt in g0_mts:
                idxs = [pl0.w1_block_of[(mt, it)] - b1_0
                        for it in pl0.g1_chains[mt]]
                w1_order.append((min(idxs), len(idxs)))
            xq = [(0, 2), (2, 4), (4, 6), (6, 9), (9, 12), (12, 16),
                  (16, 20), (20, 24)]
            xq = [(t0, t1, 0) for t0, t1 in xq]
            xi = 0

            def ship_x(n=1):
                nonlocal xi
                for _ in range(n):
                    if xi < len(xq):
                        t0, t1, c = xq[xi]
                        xi += 1
                        nc.sync.dma_start(
                            out=hin[:, t0:t1, :],
                            in_=xT[:, t0:t1, :])

            def ship_w1(s0, ns):
                if pl0.g1_fp8:
                    nc.sync.dma_start(
                        out=w1sb0[:, s0 // 2:(s0 + ns) // 2],
                        in_=w1d0[:, (b1_0 + s0) // 2:(b1_0 + s0 + ns) // 2])
                else:
                    nc.sync.dma_start(
                        out=w1sb0[:, s0 * 128:(s0 + ns) * 128],
                        in_=w1d0[:, (b1_0 + s0) * 128:(b1_0 + s0 + ns) * 128])

            for ci, (s0, ns) in enumerate(w1_order):
                ship_w1(s0, ns)
                if ci == 0:
                    ship_x(2)
                elif ci % 2 == 1:
                    ship_x()
            ship_x(len(xq))
            # w2 for group 0 is needed only ~20us in; keep it out of the
            # supply-critical x/w1 startup window
            if L1G2_NP8 > 0:
                w2d8_0, w2dB_0 = w2d0
                nc.sync.dma_start(out=w2sb0[0][:, :], in_=w2d8_0[:, 0:12])
                nb0 = 12 * (6 - NP8_2)
                nc.sync.dma_start(out=w2sb0[1][:, 0:nb0 * 64],
                                  in_=w2dB_0[:, 0:nb0 * 64])
                nc.sync.dma_start(out=w2sb0[1][:, nb0 * 64:nb0 * 128],
                                  in_=w2dB_0[:, nb0 * 64:nb0 * 128])
            else:
                h2 = b2_n // 2
                nc.sync.dma_start(
                    out=w2sb0[:, 0:h2 * 128],
                    in_=w2d0[:, b2_0 * 128:(b2_0 + h2) * 128])
                nc.sync.dma_start(
                    out=w2sb0[:, h2 * 128:b2_n * 128],
                    in_=w2d0[:, (b2_0 + h2) * 128:(b2_0 + b2_n) * 128])

            # ---- monarch layers 0..NMONARCH-1 ----
            for li in range(NMONARCH):
                pl = plans[li]

                h_dt = FP8 if (li + 1) in FP8_G1_LAYERS else ACT_DT
                hnext = sb.tile([128, pl.lout.ntiles, BPC], h_dt,
                                name=f"h_{li + 1}",
                                tag="hB" if li % 2 == 0 else "hA")

                def g1_tile(mt, mtloc, midl, w1sb, b0, css=(0, 1),
                            pl=pl, hin=hin):
                    V = int(pl.lmid.valid[mt])
                    sc = (MID8_SCALE / S1_FIXED) \
                        if (pl.li == 0 and L1G2_NP8 > 0
                            and mt % 6 < 2 * L1G2_NP8) else None
                    if sc is not None:
                        V = 128
                    its = pl.g1_chains[mt]
                    for cs in css:
                        c0 = cs * 512
                        pm = ps.tile([128, 512], F32, name=f"pm_{pl.li}",
                                     tag="pmid", bufs=3)
                        if pl.g1_fp8:
                            npair = len(its) // 2
                            for j in range(npair):
                                itA = its[2 * j]
                                assert its[2 * j + 1] == itA + 1
                                p = (pl.w1_block_of[(mt, itA)] - b0) // 2
                                nc.tensor.matmul(
                                    pm[0:V, :],
                                    w1sb[0:128, p, :, 0:V],
                                    hin[0:128, itA:itA + 2, c0:c0 + 512],
                                    start=(j == 0), stop=(j == npair - 1),
                                    perf_mode=mybir.MatmulPerfMode.DoubleRow)
                        else:
                            for j, it in enumerate(its):
                                ln = int(pl.lin.valid[it])
                                b = pl.w1_block_of[(mt, it)] - b0
                                nc.tensor.matmul(
                                    pm[0:V, :],
                                    w1sb[0:ln, b * 128:b * 128 + V],
                                    hin[0:ln, it, c0:c0 + 512],
                                    start=(j == 0),
                                    stop=(j == len(its) - 1))
                        evict(midl[0:V, mtloc, c0:c0 + 512], pm[0:V, :],
                              relu=False, scale=sc)

                def g2_tile(ot, mid_of, w2sb, b0, pl=pl, hnext=hnext):
                    # pad the output tile to all 128 rows (extra rows are
                    # matmul-computed zeros) when the next layer's fp8
                    # DoubleRow GEMM contracts whole 128-row pairs
                    if (pl.li + 1) in FP8_G1_LAYERS:
                        V = 128
                    else:
                        V = int(pl.lout.valid[ot])
                    mts = pl.g2_chains[ot]
                    fp8part = pl.li == 0 and L1G2_NP8 > 0
                    for cs in range(2):
                        c0 = cs * 512
                        po = ps.tile([128, 512], F32, name=f"po_{pl.li}",
                                     tag="pout", bufs=3)
                        if fp8part:
                            # first 2*NP8 chain blocks: fp8 DoubleRow pairs
                            # into the same PSUM accumulation as the rest
                            w2sb8, w2sbB = w2sb
                            otl = ot - b0 // 6
                            m8, _ = mid_of[mts[0]]
                            for pj in range(L1G2_NP8):
                                nc.tensor.matmul(
                                    po[0:V, :],
                                    w2sb8[0:128, otl, pj, :, 0:V],
                                    m8[0:128, 2 * pj:2 * pj + 2,
                                       c0:c0 + 512],
                                    start=(pj == 0), stop=False,
                                    perf_mode=mybir.MatmulPerfMode.DoubleRow)
                            rest = mts[2 * L1G2_NP8:]
                            nbo = 6 - 2 * L1G2_NP8
                            for j, mt in enumerate(rest):
                                ln = int(pl.lmid.valid[mt])
                                mb, loc = mid_of[mt]
                                bi = otl * nbo + j
                                nc.tensor.matmul(
                                    po[0:V, :],
                                    w2sbB[0:ln, bi * 128:bi * 128 + V],
                                    mb[0:ln, loc, c0:c0 + 512],
                                    start=False, stop=(j == len(rest) - 1))
                        else:
                            for j, mt in enumerate(mts):
                                ln = int(pl.lmid.valid[mt])
                                b = pl.w2_block_of[(ot, mt)] - b0
                                midl, loc = mid_of[mt]
                                nc.tensor.matmul(
                                    po[0:V, :],
                                    w2sb[0:ln, b * 128:b * 128 + V],
                                    midl[0:ln, loc, c0:c0 + 512],
                                    start=(j == 0),
                                    stop=(j == len(mts) - 1))
                        evict(hnext[0:V, ot, c0:c0 + 512], po[0:V, :],
                              relu=True)

                for g in range(pl.ngroups):
                    ls = list(pl.group_lset(g))
                    b1_0, b1_n = pl.group_w1range(g)
                    b2_0, b2_n = pl.group_w2range(g)

                    if li == 0 and g == 0:
                        w1sb, w2sb = w1sb0, w2sb0
                    else:
                        w1d, w2d = wdram[li]
                        if pl.g1_fp8:
                            p0, pn = b1_0 // 2, b1_n // 2
                            w1sb = sb.tile([128, pn, 2, 128], FP8,
                                           name=f"w1sb_{li}_{g}", tag="w1")
                            hp = (pn + 1) // 2
                            nc.sync.dma_start(out=w1sb[:, 0:hp],
                                              in_=w1d[:, p0:p0 + hp])
                            nc.sync.dma_start(out=w1sb[:, hp:pn],
                                              in_=w1d[:, p0 + hp:p0 + pn])
                        else:
                            w1sb = sb.tile([128, b1_n * 128], ACT_DT,
                                           name=f"w1sb_{li}_{g}", tag="w1")
                            hn = (b1_n + 1) // 2
                            nc.sync.dma_start(
                                out=w1sb[:, 0:hn * 128],
                                in_=w1d[:, b1_0 * 128:(b1_0 + hn) * 128])
                            nc.sync.dma_start(
                                out=w1sb[:, hn * 128:b1_n * 128],
                                in_=w1d[:,
                                        (b1_0 + hn) * 128:(b1_0 + b1_n) * 128])
                        if li == 0 and L1G2_NP8 > 0:
                            w2d8, w2dB = w2d
                            w2sb = (
                                sb.tile([128, 12, L1G2_NP8, 2, 128], FP8,
                                        name=f"w2sb8_{li}_{g}", tag="w28"),
                                sb.tile([128, 12 * (6 - NP8_2) * 128],
                                        ACT_DT, name=f"w2sb_{li}_{g}",
                                        tag="w2"))
                            nc.sync.dma_start(
                                out=w2sb[0][:, :],
                                in_=w2d8[:, 12 * g:12 * (g + 1)])
                            nbg = 12 * (6 - NP8_2)
                            base = g * nbg * 128
                            nc.sync.dma_start(
                                out=w2sb[1][:, 0:nbg * 64],
                                in_=w2dB[:, base:base + nbg * 64])
                            nc.sync.dma_start(
                                out=w2sb[1][:, nbg * 64:nbg * 128],
                                in_=w2dB[:, base + nbg * 64:base + nbg * 128])
                        else:
                            w2sb = sb.tile([128, b2_n * 128], ACT_DT,
                                           name=f"w2sb_{li}_{g}", tag="w2")
                            hn2 = (b2_n + 1) // 2
                            nc.sync.dma_start(
                                out=w2sb[:, 0:hn2 * 128],
                                in_=w2d[:, b2_0 * 128:(b2_0 + hn2) * 128])
                            nc.sync.dma_start(
                                out=w2sb[:, hn2 * 128:b2_n * 128],
                                in_=w2d[:,
                                        (b2_0 + hn2) * 128:(b2_0 + b2_n) * 128])

                    def alloc_mids(l, pl=pl, li=li):
                        """Per-l mid buffers; for L1 the first 2*NP8 tiles
                        live in a separate fp8 buffer."""
                        mts_l = pl.mid_tiles_of_l[l]
                        ent = {}
                        if li == 0 and L1G2_NP8 > 0:
                            n8 = 2 * L1G2_NP8
                            m8 = sb.tile([128, n8, BPC], FP8,
                                         name=f"mid8_{li}_{l}", tag="mid8",
                                         bufs=2)
                            mb = sb.tile([128, len(mts_l) - n8, BPC], ACT_DT,
                                         name=f"mid_{li}_{l}", tag="midb",
                                         bufs=2)
                            for loc, mt in enumerate(mts_l):
                                ent[mt] = (m8, loc) if loc < n8 \
                                    else (mb, loc - n8)
                        else:
                            mb = sb.tile([128, len(mts_l), BPC], ACT_DT,
                                         name=f"mid_{li}_{l}", tag="midb",
                                         bufs=2)
                            for loc, mt in enumerate(mts_l):
                                ent[mt] = (mb, loc)
                        return ent

                    if li == 0 and g == 0:
                        # interleave G1 chains across the two l's so the
                        # earliest-arriving x tiles feed as much work as
                        # possible; G2 for both l's afterwards.
                        mid_of = {}
                        for l in ls:
                            mid_of.update(alloc_mids(l))
                        for ci, mt in enumerate(g0_mts):
                            midl, loc = mid_of[mt]
                            g1_tile(mt, loc, midl, w1sb, b1_0)
                            if ci < 5:
                                pker = ps.tile([128, 256], F32,
                                               name="pker", tag="p7b",
                                               bufs=2)
                                nc.tensor.matmul(pker[:, :],
                                                 scr[0:128, 0:128],
                                                 scr[:, 0:256],
                                                 start=True, stop=True)
                        for l in ls:
                            for ot in pl.out_tiles_of_l[l]:
                                g2_tile(ot, mid_of, w2sb, b2_0)
                    elif pl.shared_mid:
                        # k-pure mid tiles shared between l's: emit each mid
                        # tile once, then the out tiles whose chains are
                        # complete (interleaved for the 8-tile layout).
                        nmt = pl.lmid.ntiles
                        if nmt == 8:
                            midA = sb.tile([128, 4, BPC], ACT_DT,
                                           name=f"mid_{li}_a", tag="midb",
                                           bufs=2)
                            midB = sb.tile([128, 4, BPC], ACT_DT,
                                           name=f"mid_{li}_b", tag="midb",
                                           bufs=2)
                            mid_of = {mt: ((midA, mt // 2) if mt % 2 == 0
                                           else (midB, mt // 2))
                                      for mt in range(nmt)}
                            order = [("m", 0), ("m", 2), ("m", 4), ("m", 6),
                                     ("m", 1), ("o", 0), ("m", 3), ("o", 1),
                                     ("m", 5), ("o", 2), ("m", 7), ("o", 3),
                                     ("o", 4), ("o", 5), ("o", 6), ("o", 7)]
                        else:
                            midA = sb.tile([128, nmt, BPC], ACT_DT,
                                           name=f"mid_{li}_a", tag="midb",
                                           bufs=2)
                            mid_of = {mt: (midA, mt) for mt in range(nmt)}
                            order = ([("m", mt) for mt in range(nmt)] +
                                     [("o", ot)
                                      for ot in range(pl.lout.ntiles)])
                        for kind, idx in order:
                            if kind == "m":
                                midl, loc = mid_of[idx]
                                g1_tile(idx, loc, midl, w1sb, b1_0)
                            else:
                                g2_tile(idx, mid_of, w2sb, b2_0)
                    else:
                        # per-l pipeline with one-block lookahead
                        mid_of = {}
                        pend = None
                        for l in ls:
                            mid_of.update(alloc_mids(l))
                            for mt in pl.mid_tiles_of_l[l]:
                                midl, loc = mid_of[mt]
                                g1_tile(mt, loc, midl, w1sb, b1_0)
                            if pend is not None:
                                for ot in pl.out_tiles_of_l[pend]:
                                    g2_tile(ot, mid_of, w2sb, b2_0)
                            pend = l
                        for ot in pl.out_tiles_of_l[pend]:
                            g2_tile(ot, mid_of, w2sb, b2_0)

                hin = hnext

            # ---- tail: dense layers + form-B logits + log_softmax,
            # pipelined over the two 512-column batch chunks so the first
            # chunk's serial DVE softmax chain overlaps the second chunk's
            # PE work ----
            from concourse.bass import broadcast_tensor_aps
            wsb_d = {}
            for li in range(NMONARCH, NLAYERS - 1):
                pl = plans[li]
                wsb_d[li] = sb.tile([128, 128 * len(pl.blocks)], ACT_DT,
                                    name=f"wd_{li}", tag="wd", bufs=3)
                nc.sync.dma_start(out=wsb_d[li][:, :], in_=wdram[li][:, :])
            w7sb = sb.tile([128, NOUT], ACT_DT, name="w7", tag="wd", bufs=3)
            nc.sync.dma_start(out=w7sb[:, :], in_=wdram[NLAYERS - 1][:, :])

            hs = {}
            hcur = hin
            for li in range(NMONARCH, NLAYERS - 1):
                pl = plans[li]
                hnext = sb.tile([128, pl.lout.ntiles, BPC], ACT_DT,
                                name=f"h_{li + 1}",
                                tag="hB" if li % 2 == 0 else "hA")
                hs[li] = (hcur, hnext)
                hcur = hnext
            h6 = hcur
            pl7 = plans[NLAYERS - 1]
            ln7 = pl7.in_valid

            nch = BPC // 128
            hc = nch // 2
            logit = sb.tile([128, nch, NOUT], F32, name="logit", tag="logit")
            esb = sb.tile([128, nch, NOUT], F32, name="esb", tag="esb")
            esum = sb.tile([128, nch], F32, name="esum", tag="esum")
            y0 = sb.tile([128, nch], F32, name="y0", tag="y0")
            ey = sb.tile([128, nch], F32, name="ey", tag="ey")
            r = sb.tile([128, nch], F32, name="r", tag="r")
            d = sb.tile([128, nch], F32, name="d", tag="d")
            s1 = sb.tile([128, nch], F32, name="s1", tag="s1")
            q = sb.tile([128, nch], F32, name="q", tag="q")
            lns = sb.tile([128, nch], F32, name="lns", tag="lns")
            osb = sb.tile([128, nch, NOUT], F32, name="osb", tag="osb")

            def dense_cs(li, cs):
                pl = plans[li]
                hi, hn = hs[li]
                c0 = cs * 512
                for ot in range(pl.lout.ntiles):
                    V = int(pl.lout.valid[ot])
                    po = ps.tile([128, 512], F32, name=f"po_{li}",
                                 tag="pout", bufs=3)
                    for j, it in enumerate(range(pl.lin.ntiles)):
                        ln = int(pl.lin.valid[it])
                        b = pl.block_of[(ot, it)]
                        nc.tensor.matmul(po[0:V, :],
                                         wsb_d[li][0:ln, b * 128:b * 128 + V],
                                         hi[0:ln, it, c0:c0 + 512],
                                         start=(j == 0),
                                         stop=(j == pl.lin.ntiles - 1))
                    evict(hn[0:V, ot, c0:c0 + 512], po[0:V, :], relu=True)

            def l7_cs(cs):
                # form-B logits for this chunk's four 128-row sub-chunks
                for ch in range(cs * hc, (cs + 1) * hc):
                    po = ps.tile([128, NOUT], F32, name="po7", tag="pout",
                                 bufs=3)
                    nc.tensor.matmul(po[:, :],
                                     h6[0:ln7, 0, ch * 128:(ch + 1) * 128],
                                     w7sb[0:ln7, :],
                                     start=True, stop=True)
                    evict(logit[:, ch, :], po[:, :], relu=False)

            def softmax_cs(cs):
                # S = sum(exp(t)); ln S via exponent-bits estimate + one
                # Newton step (only Exp needed: a single act-table set).
                cc = slice(cs * hc, (cs + 1) * hc)
                nc.scalar.activation(esb[:, cc, :], logit[:, cc, :],
                                     mybir.ActivationFunctionType.Exp)
                nc.vector.tensor_reduce(esum[:, cc], esb[:, cc, :],
                                        axis=mybir.AxisListType.X,
                                        op=mybir.AluOpType.add)
                nc.vector.tensor_scalar(y0[:, cc], esum.bitcast(I32)[:, cc],
                                        EXP_BITS_BIAS, LN2_OVER_2P23,
                                        op0=mybir.AluOpType.subtract,
                                        op1=mybir.AluOpType.mult)
                nc.scalar.activation(ey[:, cc], y0[:, cc],
                                     mybir.ActivationFunctionType.Exp,
                                     scale=-1.0)
                nc.vector.tensor_tensor(r[:, cc], esum[:, cc], ey[:, cc],
                                        op=mybir.AluOpType.mult)
                nc.vector.tensor_scalar_add(d[:, cc], r[:, cc], -1.0)
                nc.vector.tensor_tensor(s1[:, cc], d[:, cc], y0[:, cc],
                                        op=mybir.AluOpType.add)
                nc.vector.scalar_tensor_tensor(q[:, cc], d[:, cc], -0.5,
                                               d[:, cc],
                                               op0=mybir.AluOpType.mult,
                                               op1=mybir.AluOpType.mult)
                nc.vector.tensor_tensor(lns[:, cc], s1[:, cc], q[:, cc],
                                        op=mybir.AluOpType.add)
                lg_ap, ln_ap = broadcast_tensor_aps(
                    logit[:, cc, :],
                    lns[:, cc].rearrange("p (c u) -> p c u", u=1))
                nc.vector.tensor_tensor(osb[:, cc, :], lg_ap, ln_ap,
                                        op=mybir.AluOpType.subtract)
                nc.sync.dma_start(out=y[:, cc, :], in_=osb[:, cc, :])

            # dense layers run both chunks back-to-back; the chunk-0 softmax
            # DVE chain then overlaps chunk 1's form-B PE work.
            for li in range(NMONARCH, NLAYERS - 1):
                dense_cs(li, 0)
                dense_cs(li, 1)
            l7_cs(0)
            softmax_cs(0)
            l7_cs(1)
            softmax_cs(1)
    nc.finalize()
    return nc


# ------------------------------------------------------------------ entry point
def _prep_inputs(inputs, plans):
    np_dt = mybir.dt.np(ACT_DT)
    np_dt8 = mybir.dt.np(FP8)
    x_np_dt = np_dt8 if L1G1_FP8 else np_dt
    x = np.ascontiguousarray(np.asarray(inputs["x"], dtype=np.float32))
    shared = {}
    for i, pl in enumerate(plans):
        w1 = np.asarray(inputs[f"w1_{i + 1}"], dtype=np.float32)
        w2 = np.asarray(inputs[f"w2_{i + 1}"], dtype=np.float32)
        if isinstance(pl, LayerPlan):
            W1m, W2m = pl.build_weights(w1, w2)
            w1_dt = np_dt8 if pl.g1_fp8 else np_dt
            shared[f"w1c_{i}"] = np.ascontiguousarray(W1m.astype(w1_dt))
            if isinstance(W2m, tuple):
                W2m8, W2mB = W2m
                shared[f"w2c8_{i}"] = np.ascontiguousarray(
                    W2m8.astype(np_dt8))
                shared[f"w2c_{i}"] = np.ascontiguousarray(W2mB.astype(np_dt))
            else:
                shared[f"w2c_{i}"] = np.ascontiguousarray(W2m.astype(np_dt))
        elif isinstance(pl, DensePlan):
            Wm = pl.build_weights(w1, w2)
            shared[f"wdc_{i}"] = np.ascontiguousarray(Wm.astype(np_dt))
        else:
            W = pl.build_weights(w1, w2)
            shared[f"w7c_{i}"] = np.ascontiguousarray(W.astype(np_dt))
    in_maps = []
    for c in range(NCORES):
        m = dict(shared)
        xc = x[c * BPC:(c + 1) * BPC].T.astype(x_np_dt)    # [3072, 1024]
        m["xT"] = np.ascontiguousarray(
            xc.reshape(plans[0].lin.ntiles, 128, BPC).transpose(1, 0, 2))
        in_maps.append(m)
    return in_maps


def _run(inputs, trace=False, **spmd_kwargs):
    plans = build_plans()
    in_maps = _prep_inputs(inputs, plans)
    nc = build_program(plans)
    res = run_bass_kernel_spmd(nc, in_maps, core_ids=list(range(NCORES)),
                               trace=trace, **spmd_kwargs)
    nch = BPC // 128
    out = np.concatenate(
        [np.asarray(r["y"]).reshape(128, nch, NOUT)
         .transpose(1, 0, 2).reshape(BPC, NOUT)
         for r in res.results], axis=0)
    return out.astype(np.float32), res


def kernel(**inputs):
    out, _ = _run(inputs, trace=False)
    return out



# revision 54
# speedup vs baseline: 1.0040x; 1.0040x over previous
"""Trainium2 Bass kernel for the CIFAR10 Monarch MLP (7 monarch layers + log_softmax).

Strategy
--------
Pure data parallel over 8 NeuronCores: each core takes a 1024-row batch shard;
the ~9M-param block-diagonal weights are replicated.

On-device dataflow is feature-major: activations live in SBUF as
[features (128-partition tiles), batch (free dim)], fully SBUF-resident across
all layers; only x, the weights and the final log-probs cross HBM.

Layers 1-4 keep the monarch two-GEMM structure expressed as block-sparse
matmuls over the *effective* weight matrices (butterfly permutation folded
into W1 on the host).  The mid layouts are chosen to minimise 128x128 tile
count: layers 1-2 group mid features by output block l with k-aligned
sub-groups; layers 3-4 use k-pure mid tiles (every mid tile draws from one
input block, with 2 or 4 l-cells packed per tile and the unused rows made
zero by construction), which cuts L3 from 80 to 64 and L4 from 32 to 24
tiles.  Layers 5-6 are fused into a single dense GEMM each.  Layer 7 is
fused and computed batch-major (activation tile as the stationary operand),
so the logits land as [batch, 10] in PSUM and log_softmax needs no
transposes.

GEMM1 of layers 1-2 runs in fp8-e4m3 DoubleRow mode: two 128-row
contraction tiles per instruction at 2x PE rate (157 TF/s).  Each W1 is
pre-scaled by a power of two to rms~2 for e4m3 and the inverse scale is
folded into that layer's bf16 W2, so no extra runtime ops appear; layer 1
reads fp8 x straight from HBM and layer 1's G2 evicts PSUM->fp8 (padded to
full 128-row tiles so DoubleRow contracts whole pairs).  Layer 1's GEMM2
additionally contracts the first two mid tiles of each l-chain as one fp8
DoubleRow pair accumulated into the same PSUM as the remaining bf16
blocks: those mid tiles are evicted as fp8 at rms~0.19 and the fp8 W2 part
carries the inverse scale, so the scale product is exactly 1 and no fixup
op is needed.  Full fp8 on any further GEMM was measured (host-sim + HW)
to push max rel err past the 2e-2 gate; this config lands at 1.59e-2 vs
2.3e-3 all-bf16.

The tensor engine needs ~3us of continuous work to reach max clock, so the
startup burns the DMA wait with dummy 256-row matmuls; a mid-stream PE gap
resets the ramp, so the warm-up is sized to hand over to real work just as
the first x/W1 slices land.  The tail pipelines dense layers / form-B /
softmax over the two 512-column batch chunks so the serial DVE softmax
chain overlaps PE work, and y is written in on-chip [partition, chunk,
class] order (the host unpermutes) to avoid a 1024-descriptor scatter DMA.

log_softmax avoids the Ln activation function entirely: ln(S) is computed
with a DVE exponent-extraction estimate refined by one Newton step that only
needs Exp.  Every activation function used (Copy / Relu / Exp) lives in one
activation-table set, so the program loads act tables exactly once - extra
InstLoadActFuncSet switches measurably slow *all* PE matmuls by ~20%.
"""

import os as _os

import numpy as np

import concourse.bacc as bacc_mod
import concourse.mybir as mybir
import concourse.tile as tile
from concourse.bass_utils import run_bass_kernel_spmd

# ----------------------------------------------------------------- problem dims
BATCH = 8192
IN_FEATURES = 3072
NCORES = 8
BPC = BATCH // NCORES          # 1024 batch rows per core
NOUT = 10

SHAPES = [((4, 750, 768), (4, 750, 750)),
          ((4, 500, 750), (4, 500, 500)),
          ((4, 250, 500), (4, 250, 250)),
          ((4, 125, 250), (4, 125, 125)),
          ((4, 50, 125), (4, 50, 50)),
          ((4, 25, 50), (4, 25, 25)),
          ((4, 3, 25), (4, 3, 3))]
NLAYERS = 7
NMONARCH = 4                   # layers 0..3 stay factored; 4..5 dense; 6 form-B

F32 = mybir.dt.float32
I32 = mybir.dt.int32
FP8 = mybir.dt.float8e4
ACT_DT_NAME = _os.environ.get("KERNEL_MM_DT", "bf16")
ACT_DT = {"fp32": mybir.dt.float32,
          "fp32r": mybir.dt.float32r,
          "bf16": mybir.dt.bfloat16}[ACT_DT_NAME]
# GEMM1 of layers 1-2 runs in fp8-e4m3 DoubleRow mode (2 contraction tiles
# per instruction at 2x rate).  Each W1 is pre-scaled to unit-ish rms for
# e4m3 and the inverse scale is folded into that layer's (bf16) W2, so no
# extra runtime ops appear.  Layer 1's activations enter fp8 from HBM;
# layer 2's are evicted PSUM->fp8 by layer 1's G2 (padded to full 128-row
# tiles so DoubleRow can contract whole pairs).
L1G1_FP8 = _os.environ.get("KERNEL_L1G1_FP8", "1") == "1"
FP8_G1_LAYERS = (0, 1) if L1G1_FP8 else ()
# Partial fp8 on L1-GEMM2: the first 2*NP8 mid tiles of each l-chain are
# evicted as fp8 (scaled to rms ~0.19) and contracted as DoubleRow pairs
# into the same PSUM accumulation as the remaining bf16 blocks.  The scale
# product of the fp8 W2 part and mid8 is 1, so no fixup op is needed.
L1G2_NP8 = int(_os.environ.get("KERNEL_L1G2_NP8", "1")) if L1G1_FP8 else 0
S1_FIXED = 64.0          # W1 fp8 scale for layers 1-2 (randn/sqrt(750ish))
MID8_SCALE = 0.1875      # mid8 = mid_true * MID8_SCALE (rms ~0.19)

LN2_OVER_2P23 = float(np.log(2.0) / (1 << 23))   # 8.262958e-08
EXP_BITS_BIAS = 1065353216.0                     # bitcast(1.0f)


# ------------------------------------------------------------------ layouts
class Layout:
    """Placement of 4 feature blocks of size Sb into 128-partition tiles."""

    @classmethod
    def from_positions(cls, Sb, ntiles, feat_tile, feat_row):
        self = object.__new__(cls)
        self.Sb = Sb
        self.ntiles = ntiles
        self.feat_tile = feat_tile
        self.feat_row = feat_row
        self._finish()
        return self

    def _finish(self):
        self.valid = np.zeros(self.ntiles, np.int64)
        for k in range(4):
            for t, r in zip(self.feat_tile[k], self.feat_row[k]):
                self.valid[t] = max(self.valid[t], r + 1)
        self.grow = [self.feat_tile[k] * 128 + self.feat_row[k]
                     for k in range(4)]
        self.tiles_of_block = [sorted(set(self.feat_tile[k].tolist()))
                               for k in range(4)]


def simple_layout(Sb):
    """Blocks >= 128 rows: chunked over dedicated tiles. 65..127: one tile
    each. <= 64: packed at 32-aligned offsets."""
    if Sb >= 128:
        cpb = (Sb + 127) // 128
        ft, fr = [], []
        for k in range(4):
            i = np.arange(Sb)
            ft.append(k * cpb + i // 128)
            fr.append(i % 128)
        return Layout.from_positions(Sb, 4 * cpb, ft, fr)
    stride = ((Sb + 31) // 32) * 32
    bpt = max(1, 128 // stride)
    ntiles = (4 + bpt - 1) // bpt
    ft, fr = [], []
    for k in range(4):
        i = np.arange(Sb)
        ft.append(np.full(Sb, k // bpt, np.int64))
        fr.append((k % bpt) * stride + i)
    return Layout.from_positions(Sb, ntiles, ft, fr)


def grouped_mid_layout(R, Q):
    """Mid layout with features regrouped by input block k (R >= 125)."""
    cpb = max(1, (R + 127) // 128)
    block_rows = cpb * 128
    Gp = block_rows // 4
    ft, fr = [], []
    for l in range(4):
        rs = np.arange(R)
        ks = (4 * rs + l) // Q
        pos = np.empty(R, np.int64)
        for k in range(4):
            idx = rs[ks == k]
            assert len(idx) <= Gp
            pos[idx] = k * Gp + np.arange(len(idx))
        ft.append(l * cpb + pos // 128)
        fr.append(pos % 128)
    return Layout.from_positions(R, 4 * cpb, ft, fr)


def kpure_mid_layout(R, Q, slot):
    """k-pure mid tiles: every tile draws from a single input block k, so a
    G1 chain needs only that k's input tiles.  slot=64: two l's per tile
    (tile k*2 + l//2, cells at 64-row offsets; needs cells <= 64).  slot=32:
    one tile per k holding all four l cells at 32-row offsets (cells <= 32)."""
    ft, fr = [], []
    ntiles = 8 if slot == 64 else 4
    for l in range(4):
        rs = np.arange(R)
        ks = (4 * rs + l) // Q
        pos_t = np.empty(R, np.int64)
        pos_r = np.empty(R, np.int64)
        for k in range(4):
            idx = rs[ks == k]
            assert len(idx) <= slot
            if slot == 64:
                pos_t[idx] = k * 2 + l // 2
                pos_r[idx] = (l % 2) * 64 + np.arange(len(idx))
            else:
                pos_t[idx] = k
                pos_r[idx] = l * 32 + np.arange(len(idx))
        ft.append(pos_t)
        fr.append(pos_r)
    return Layout.from_positions(R, ntiles, ft, fr)


def build_full_mats(w1_shape, w2_shape, lin, lmid, lout):
    """Shapes of the dense effective matrices (values filled on the host)."""
    return ((lin.ntiles * 128, lmid.ntiles * 128),
            (lmid.ntiles * 128, lout.ntiles * 128))


class LayerPlan:
    """Factored (monarch) layer: two block-sparse GEMMs."""

    def __init__(self, li, w1_shape, w2_shape, in_layout):
        _, Q, P = w1_shape
        _, S, R = w2_shape
        self.li, self.P, self.Q, self.R, self.S = li, P, Q, R, S
        self.lin = in_layout
        cell_max = max(
            int(np.sum((4 * np.arange(R) + l) // Q == k))
            for l in range(4) for k in range(4))
        if cell_max <= 32:
            self.lmid = kpure_mid_layout(R, Q, 32)
        elif cell_max <= 64:
            self.lmid = kpure_mid_layout(R, Q, 64)
        elif R >= 125:
            self.lmid = grouped_mid_layout(R, Q)
        else:
            self.lmid = simple_layout(R)
        self.shared_mid = cell_max <= 64
        self.lout = simple_layout(S)
        self.ngroups = 2 if li == 0 else 1
        self._build()

    def _build(self):
        Q, R, S = self.Q, self.R, self.S
        ks_of = [(4 * np.arange(R) + l) // Q for l in range(4)]

        need1 = {}
        for l in range(4):
            for r in range(R):
                mt = int(self.lmid.feat_tile[l][r])
                k = int(ks_of[l][r])
                need1.setdefault(mt, set()).update(self.lin.tiles_of_block[k])
        self.g1_chains = {mt: sorted(its) for mt, its in need1.items()}
        self.w1_blocks = [(mt, it) for mt in sorted(need1)
                          for it in self.g1_chains[mt]]
        self.w1_block_of = {p: i for i, p in enumerate(self.w1_blocks)}

        need2 = {}
        for l in range(4):
            for s in range(S):
                ot = int(self.lout.feat_tile[l][s])
                need2.setdefault(ot, set()).update(self.lmid.tiles_of_block[l])
        self.g2_chains = {ot: sorted(mts) for ot, mts in need2.items()}
        self.w2_blocks = [(ot, mt) for ot in sorted(need2)
                          for mt in self.g2_chains[ot]]
        self.w2_block_of = {p: i for i, p in enumerate(self.w2_blocks)}

        self.mid_tiles_of_l = [self.lmid.tiles_of_block[l] for l in range(4)]
        self.out_tiles_of_l = [self.lout.tiles_of_block[l] for l in range(4)]

    def group_lset(self, g):
        return range(4) if self.ngroups == 1 else range(2 * g, 2 * g + 2)

    def group_w1range(self, g):
        mts = {t for l in self.group_lset(g) for t in self.mid_tiles_of_l[l]}
        idxs = [i for i, (mt, _) in enumerate(self.w1_blocks) if mt in mts]
        assert idxs == list(range(idxs[0], idxs[0] + len(idxs)))
        return idxs[0], len(idxs)

    def group_w2range(self, g):
        ots = {t for l in self.group_lset(g) for t in self.out_tiles_of_l[l]}
        idxs = [i for i, (ot, _) in enumerate(self.w2_blocks) if ot in ots]
        assert idxs == list(range(idxs[0], idxs[0] + len(idxs)))
        return idxs[0], len(idxs)

    def full_mats(self, w1, w2):
        """Dense effective matrices (butterfly folded into W1)."""
        Q, R = self.Q, self.R
        W1full = np.zeros((self.lin.ntiles * 128, self.lmid.ntiles * 128),
                          np.float32)
        W2full = np.zeros((self.lmid.ntiles * 128, self.lout.ntiles * 128),
                          np.float32)
        for l in range(4):
            js = 4 * np.arange(R) + l
            ks, qs = js // Q, js % Q
            mcols = self.lmid.grow[l]
            for k in range(4):
                sel = np.where(ks == k)[0]
                if len(sel) == 0:
                    continue
                W1full[np.ix_(self.lin.grow[k], mcols[sel])] = \
                    np.ascontiguousarray(w1[k, qs[sel], :].T)
            W2full[np.ix_(self.lmid.grow[l], self.lout.grow[l])] = \
                np.ascontiguousarray(w2[l].T)
        return W1full, W2full

    @property
    def g1_fp8(self):
        return self.li in FP8_G1_LAYERS

    def build_weights(self, w1, w2):
        """Host: gather the nonzero 128x128 tiles into [128, nblocks*128].
        For the fp8 layer, W1 is scaled by a power of two to rms~2 (e4m3
        sweet spot), quantized, and packed as DoubleRow pairs
        [128, npairs, 2, 128]; W2 absorbs the inverse scale."""
        W1full, W2full = self.full_mats(w1, w2)
        W1m = np.zeros((128, 128 * len(self.w1_blocks)), np.float32)
        for i, (mt, it) in enumerate(self.w1_blocks):
            W1m[:, i * 128:(i + 1) * 128] = \
                W1full[it * 128:(it + 1) * 128, mt * 128:(mt + 1) * 128]
        W2m = np.zeros((128, 128 * len(self.w2_blocks)), np.float32)
        for i, (ot, mt) in enumerate(self.w2_blocks):
            W2m[:, i * 128:(i + 1) * 128] = \
                W2full[mt * 128:(mt + 1) * 128, ot * 128:(ot + 1) * 128]
        if self.g1_fp8:
            s = S1_FIXED
            W1m = (W1m * s).reshape(128, len(self.w1_blocks) // 2, 2, 128)
            W2m = W2m * (1.0 / s)
        if self.li == 0 and L1G2_NP8 > 0:
            # split W2 per out tile: first 2*NP8 chain blocks become fp8
            # DoubleRow pairs (scaled 1/MID8_SCALE to pair with mid8 so the
            # product lands at natural scale in the shared PSUM), rest bf16.
            # W2m rows here already carry the 1/s fold; undo it for the fp8
            # part since mid8 is scaled from mid_true, not s*mid_true.
            n8 = 2 * L1G2_NP8
            not_ = len(self.g2_chains)
            W2m8 = np.zeros((128, not_, L1G2_NP8, 2, 128), np.float32)
            W2mB = np.zeros((128, not_ * (6 - n8) * 128), np.float32)
            for i, (ot, mt) in enumerate(self.w2_blocks):
                j = i % 6
                blk = W2m[:, i * 128:(i + 1) * 128]
                if j < n8:
                    W2m8[:, ot, j // 2, j % 2, :] = \
                        blk * (S1_FIXED / MID8_SCALE)
                else:
                    bi = ot * (6 - n8) + (j - n8)
                    W2mB[:, bi * 128:(bi + 1) * 128] = blk
            return W1m, (W2m8, W2mB)
        return W1m, W2m


class DensePlan:
    """Fused layer: one dense GEMM over the product W1eff @ W2eff."""

    def __init__(self, li, w1_shape, w2_shape, in_layout):
        self.li = li
        self.fact = LayerPlan(li, w1_shape, w2_shape, in_layout)
        self.lin = in_layout
        self.lout = self.fact.lout
        self.blocks = [(ot, it)
                       for ot in range(self.lout.ntiles)
                       for it in range(self.lin.ntiles)]
        self.block_of = {p: i for i, p in enumerate(self.blocks)}

    def build_weights(self, w1, w2):
        W1full, W2full = self.fact.full_mats(w1, w2)
        Wd = W1full @ W2full
        Wm = np.zeros((128, 128 * len(self.blocks)), np.float32)
        for i, (ot, it) in enumerate(self.blocks):
            Wm[:, i * 128:(i + 1) * 128] = \
                Wd[it * 128:(it + 1) * 128, ot * 128:(ot + 1) * 128]
        return Wm


class FormBPlan:
    """Final layer: fused dense [in_rows x NOUT], computed batch-major with
    the activation tile as the stationary operand."""

    def __init__(self, li, w1_shape, w2_shape, in_layout):
        self.li = li
        self.fact = LayerPlan(li, w1_shape, w2_shape, in_layout)
        self.lin = in_layout
        assert self.lin.ntiles == 1
        self.in_valid = int(self.lin.valid[0])

    def build_weights(self, w1, w2):
        W1full, W2full = self.fact.full_mats(w1, w2)
        Wd = W1full @ W2full                       # [in_rows, out_grow cols]
        lout = self.fact.lout
        cols = [int(lout.grow[n // 3][n % 3]) for n in range(NOUT)]
        W = np.zeros((128, NOUT), np.float32)
        W[:self.in_valid + 0, :] = Wd[:self.in_valid, cols][: 128]
        return W


def build_plans():
    plans = []
    lin = simple_layout(SHAPES[0][0][2])
    for i, (s1, s2) in enumerate(SHAPES):
        if i < NMONARCH:
            pl = LayerPlan(i, s1, s2, lin)
        elif i < NLAYERS - 1:
            pl = DensePlan(i, s1, s2, lin)
        else:
            pl = FormBPlan(i, s1, s2, lin)
        plans.append(pl)
        lin = pl.lout if i < NLAYERS - 1 else None
    return plans


# --------------------------------------------------- numpy model of the schedule
def numpy_forward(plans, weights, xT):
    B = xT.shape[1]
    h = np.zeros((plans[0].lin.ntiles * 128, B), np.float32)
    h[:xT.shape[0]] = xT
    for pl in plans:
        if isinstance(pl, LayerPlan):
            W1m, W2m = weights[pl.li]
            mid = np.zeros((pl.lmid.ntiles * 128, B), np.float32)
            for mt, its in pl.g1_chains.items():
                V = pl.lmid.valid[mt]
                acc = np.zeros((V, B), np.float32)
                for it in its:
                    ln = pl.lin.valid[it]
                    b = pl.w1_block_of[(mt, it)]
                    acc += W1m[0:ln, b * 128:b * 128 + V].T @ \
                        h[it * 128: it * 128 + ln]
                mid[mt * 128: mt * 128 + V] = acc
            out = np.zeros((pl.lout.ntiles * 128, B), np.float32)
            for ot, mts in pl.g2_chains.items():
                V = pl.lout.valid[ot]
                acc = np.zeros((V, B), np.float32)
                for mt in mts:
                    ln = pl.lmid.valid[mt]
                    b = pl.w2_block_of[(ot, mt)]
                    acc += W2m[0:ln, b * 128:b * 128 + V].T @ \
                        mid[mt * 128: mt * 128 + ln]
                out[ot * 128: ot * 128 + V] = acc
            h = np.maximum(out, 0.0)
        elif isinstance(pl, DensePlan):
            Wm = weights[pl.li]
            out = np.zeros((pl.lout.ntiles * 128, B), np.float32)
            for ot in range(pl.lout.ntiles):
                V = pl.lout.valid[ot]
                acc = np.zeros((V, B), np.float32)
                for it in range(pl.lin.ntiles):
                    ln = pl.lin.valid[it]
                    b = pl.block_of[(ot, it)]
                    acc += Wm[0:ln, b * 128:b * 128 + V].T @ \
                        h[it * 128: it * 128 + ln]
                out[ot * 128: ot * 128 + V] = acc
            h = np.maximum(out, 0.0)
        else:
            W = weights[pl.li]                     # [128, NOUT]
            ln = pl.in_valid
            logits = h[0:ln, :].T @ W[0:ln, :]     # [B, NOUT]
            t = logits
            s = np.exp(t).sum(axis=1, keepdims=True)
            return t - np.log(s)
    raise AssertionError


# ------------------------------------------------------------------ bass program
def build_program(plans):
    nc = bacc_mod.Bacc()

    x_dt = FP8 if L1G1_FP8 else ACT_DT
    # partition-major x in HBM: each DMA slice is one contiguous run per
    # partition (vs one run per tile), cutting startup descriptor count
    xT = nc.dram_tensor("xT", [128, plans[0].lin.ntiles, BPC], x_dt,
                        kind="ExternalInput")
    wdram = []
    for i, p in enumerate(plans):
        if isinstance(p, LayerPlan):
            if p.g1_fp8:
                w1t = nc.dram_tensor(
                    f"w1c_{i}", [128, len(p.w1_blocks) // 2, 2, 128],
                    FP8, kind="ExternalInput")
            else:
                w1t = nc.dram_tensor(
                    f"w1c_{i}", [128, 128 * len(p.w1_blocks)],
                    ACT_DT, kind="ExternalInput")
            if i == 0 and L1G2_NP8 > 0:
                n8 = 2 * L1G2_NP8
                w2t = (
                    nc.dram_tensor(f"w2c8_{i}",
                                   [128, len(p.g2_chains), L1G2_NP8, 2, 128],
                                   FP8, kind="ExternalInput"),
                    nc.dram_tensor(f"w2c_{i}",
                                   [128, len(p.g2_chains) * (6 - n8) * 128],
                                   ACT_DT, kind="ExternalInput"))
            else:
                w2t = nc.dram_tensor(f"w2c_{i}",
                                     [128, 128 * len(p.w2_blocks)],
                                     ACT_DT, kind="ExternalInput")
            wdram.append((w1t, w2t))
        elif isinstance(p, DensePlan):
            wdram.append(nc.dram_tensor(f"wdc_{i}", [128, 128 * len(p.blocks)],
                                        ACT_DT, kind="ExternalInput"))
        else:
            wdram.append(nc.dram_tensor(f"w7c_{i}", [128, NOUT], ACT_DT,
                                        kind="ExternalInput"))
    # y stays in the on-chip [partition, chunk, class] order; the host
    # unpermutes (batch row = chunk*128 + partition).  A [BPC, NOUT] dram
    # layout costs ~8us at kernel end: 1024 scattered 40-byte descriptors.
    y = nc.dram_tensor("y", [128, BPC // 128, NOUT], F32, kind="ExternalOutput")

    with tile.TileContext(nc) as tc:
        with (
            tc.tile_pool(name="sb", bufs=1) as sb,
            tc.tile_pool(name="ps", bufs=1, space="PSUM") as ps,
        ):
            evict_flip = [0]

            def evict(dst_ap, src_ap, relu, scale=None):
                e = evict_flip[0] = evict_flip[0] ^ 1
                if scale is not None:
                    if e:
                        nc.vector.tensor_scalar_mul(dst_ap, src_ap, scale)
                    else:
                        nc.scalar.activation(
                            dst_ap, src_ap,
                            mybir.ActivationFunctionType.Copy, scale=scale)
                elif relu:
                    if e:
                        nc.vector.tensor_scalar_max(dst_ap, src_ap, 0.0)
                    else:
                        nc.scalar.activation(dst_ap, src_ap,
                                             mybir.ActivationFunctionType.Relu)
                else:
                    if e:
                        nc.vector.tensor_copy(dst_ap, src_ap)
                    else:
                        nc.scalar.copy(dst_ap, src_ap)

            # ---- PE p-state warm-up: the tensor engine needs ~3us of
            # continuous work to reach max clock, and the first real chains
            # otherwise run 2x slow while DMA still streams x/weights.  Burn
            # the idle startup window with dummy matmuls on a zeroed scratch
            # tile (results discarded).  memset on gpsimd (idle and ready
            # ~1.5us before the vector engine at program start) and 256-row
            # warm-ups so the ramp completes with minimal overshoot.
            scr = sb.tile([128, 256], ACT_DT, name="scr", tag="scr")
            nc.gpsimd.memset(scr[:, :], 0.0)
            pwarm = ps.tile([128, 256], F32, name="pwarm", tag="p7b", bufs=2)
            for _ in range(16):
                nc.tensor.matmul(pwarm[:, :], scr[0:128, 0:128], scr[:, 0:256],
                                 start=True, stop=True)

            # ---- startup: first weight slices before/interleaved with x ----
            pl0 = plans[0]
            b1_0, b1_n = pl0.group_w1range(0)
            b2_0, b2_n = pl0.group_w2range(0)
            if pl0.g1_fp8:
                w1sb0 = sb.tile([128, b1_n // 2, 2, 128], FP8,
                                name="w1sb_0_0", tag="w1")
            else:
                w1sb0 = sb.tile([128, b1_n * 128], ACT_DT, name="w1sb_0_0",
                                tag="w1")
            NP8_2 = 2 * L1G2_NP8
            if L1G2_NP8 > 0:
                w2sb0 = (
                    sb.tile([128, 12, L1G2_NP8, 2, 128], FP8,
                            name="w2sb8_0_0", tag="w28"),
                    sb.tile([128, 12 * (6 - NP8_2) * 128], ACT_DT,
                            name="w2sb_0_0", tag="w2"))
            else:
                w2sb0 = sb.tile([128, b2_n * 128], ACT_DT, name="w2sb_0_0",
                                tag="w2")
            hin = sb.tile([128, pl0.lin.ntiles, BPC], x_dt,
                          name="h_in0", tag="hA")

            w1d0, w2d0 = wdram[0]
            # G1 chains of group 0 are emitted interleaved across l=0,1 (see
            # below); ship weight slices in that order, interleaved with x.
            g0_mts = []
            for a, b in zip(pl0.mid_tiles_of_l[0], pl0.mid_tiles_of_l[1]):
                g0_mts.extend((a, b))
            w1_order = []        # (block_start, block_count) per chain
            for mt in g0_mts:
                idxs = [pl0.w1_block_of[(mt, it)] - b1_0
                        for it in pl0.g1_chains[mt]]
                w1_order.append((min(idxs), len(idxs)))
            xq = [(0, 2), (2, 4), (4, 6), (6, 9), (9, 12), (12, 16),
                  (16, 20), (20, 24)]
            xq = [(t0, t1, 0) for t0, t1 in xq]
            xi = 0

            def ship_x(n=1):
                nonlocal xi
                for _ in range(n):
                    if xi < len(xq):
                        t0, t1, c = xq[xi]
                        xi += 1
                        nc.sync.dma_start(
                            out=hin[:, t0:t1, :],
                            in_=xT[:, t0:t1, :])

            def ship_w1(s0, ns):
                if pl0.g1_fp8:
                    nc.sync.dma_start(
                        out=w1sb0[:, s0 // 2:(s0 + ns) // 2],
                        in_=w1d0[:, (b1_0 + s0) // 2:(b1_0 + s0 + ns) // 2])
                else:
                    nc.sync.dma_start(
                        out=w1sb0[:, s0 * 128:(s0 + ns) * 128],
                        in_=w1d0[:, (b1_0 + s0) * 128:(b1_0 + s0 + ns) * 128])

            for ci, (s0, ns) in enumerate(w1_order):
                ship_w1(s0, ns)
                if ci == 0:
                    ship_x(2)
                elif ci % 2 == 1:
                    ship_x()
            ship_x(len(xq))
            # w2 for group 0 is needed only ~20us in; keep it out of the
            # supply-critical x/w1 startup window
            if L1G2_NP8 > 0:
                w2d8_0, w2dB_0 = w2d0
                nc.sync.dma_start(out=w2sb0[0][:, :], in_=w2d8_0[:, 0:12])
                nb0 = 12 * (6 - NP8_2)
                nc.sync.dma_start(out=w2sb0[1][:, 0:nb0 * 64],
                                  in_=w2dB_0[:, 0:nb0 * 64])
                nc.sync.dma_start(out=w2sb0[1][:, nb0 * 64:nb0 * 128],
                                  in_=w2dB_0[:, nb0 * 64:nb0 * 128])
            else:
                h2 = b2_n // 2
                nc.sync.dma_start(
                    out=w2sb0[:, 0:h2 * 128],
                    in_=w2d0[:, b2_0 * 128:(b2_0 + h2) * 128])
                nc.sync.dma_start(
                    out=w2sb0[:, h2 * 128:b2_n * 128],
                    in_=w2d0[:, (b2_0 + h2) * 128:(b2_0 + b2_n) * 128])

            # ---- monarch layers 0..NMONARCH-1 ----
            for li in range(NMONARCH):
                pl = plans[li]

                h_dt = FP8 if (li + 1) in FP8_G1_LAYERS else ACT_DT
                hnext = sb.tile([128, pl.lout.ntiles, BPC], h_dt,
                                name=f"h_{li + 1}",
                                tag="hB" if li % 2 == 0 else "hA")

                def g1_tile(mt, mtloc, midl, w1sb, b0, css=(0, 1),
                            pl=pl, hin=hin):
                    V = int(pl.lmid.valid[mt])
                    sc = (MID8_SCALE / S1_FIXED) \
                        if (pl.li == 0 and L1G2_NP8 > 0
                            and mt % 6 < 2 * L1G2_NP8) else None
                    if sc is not None:
                        V = 128
                    its = pl.g1_chains[mt]
                    for cs in css:
                        c0 = cs * 512
                        pm = ps.tile([128, 512], F32, name=f"pm_{pl.li}",
                                     tag="pmid", bufs=3)
                        if pl.g1_fp8:
                            npair = len(its) // 2
                            for j in range(npair):
                                itA = its[2 * j]
                                assert its[2 * j + 1] == itA + 1
                                p = (pl.w1_block_of[(mt, itA)] - b0) // 2
                                nc.tensor.matmul(
                                    pm[0:V, :],
                                    w1sb[0:128, p, :, 0:V],
                                    hin[0:128, itA:itA + 2, c0:c0 + 512],
                                    start=(j == 0), stop=(j == npair - 1),
                                    perf_mode=mybir.MatmulPerfMode.DoubleRow)
                        else:
                            for j, it in enumerate(its):
                                ln = int(pl.lin.valid[it])
                                b = pl.w1_block_of[(mt, it)] - b0
                                nc.tensor.matmul(
                                    pm[0:V, :],
                                    w1sb[0:ln, b * 128:b * 128 + V],
                                    hin[0:ln, it, c0:c0 + 512],
                                    start=(j == 0),
                                    stop=(j == len(its) - 1))
                        evict(midl[0:V, mtloc, c0:c0 + 512], pm[0:V, :],
                              relu=False, scale=sc)

                def g2_tile(ot, mid_of, w2sb, b0, pl=pl, hnext=hnext):
                    # pad the output tile to all 128 rows (extra rows are
                    # matmul-computed zeros) when the next layer's fp8
                    # DoubleRow GEMM contracts whole 128-row pairs
                    if (pl.li + 1) in FP8_G1_LAYERS:
                        V = 128
                    else:
                        V = int(pl.lout.valid[ot])
                    mts = pl.g2_chains[ot]
                    fp8part = pl.li == 0 and L1G2_NP8 > 0
                    for cs in range(2):
                        c0 = cs * 512
                        po = ps.tile([128, 512], F32, name=f"po_{pl.li}",
                                     tag="pout", bufs=3)
                        if fp8part:
                            # first 2*NP8 chain blocks: fp8 DoubleRow pairs
                            # into the same PSUM accumulation as the rest
                            w2sb8, w2sbB = w2sb
                            otl = ot - b0 // 6
                            m8, _ = mid_of[mts[0]]
                            for pj in range(L1G2_NP8):
                                nc.tensor.matmul(
                                    po[0:V, :],
                                    w2sb8[0:128, otl, pj, :, 0:V],
                                    m8[0:128, 2 * pj:2 * pj + 2,
                                       c0:c0 + 512],
                                    start=(pj == 0), stop=False,
                                    perf_mode=mybir.MatmulPerfMode.DoubleRow)
                            rest = mts[2 * L1G2_NP8:]
                            nbo = 6 - 2 * L1G2_NP8
                            for j, mt in enumerate(rest):
                                ln = int(pl.lmid.valid[mt])
                                mb, loc = mid_of[mt]
                                bi = otl * nbo + j
                                nc.tensor.matmul(
                                    po[0:V, :],
                                    w2sbB[0:ln, bi * 128:bi * 128 + V],
                                    mb[0:ln, loc, c0:c0 + 512],
                                    start=False, stop=(j == len(rest) - 1))
                        else:
                            for j, mt in enumerate(mts):
                                ln = int(pl.lmid.valid[mt])
                                b = pl.w2_block_of[(ot, mt)] - b0
                                midl, loc = mid_of[mt]
                                nc.tensor.matmul(
                                    po[0:V, :],
                                    w2sb[0:ln, b * 128:b * 128 + V],
                                    midl[0:ln, loc, c0:c0 + 512],
                                    start=(j == 0),
                                    stop=(j == len(mts) - 1))
                        evict(hnext[0:V, ot, c0:c0 + 512], po[0:V, :],
                              relu=True)

                for g in range(pl.ngroups):
                    ls = list(pl.group_lset(g))
                    b1_0, b1_n = pl.group_w1range(g)
                    b2_0, b2_n = pl.group_w2range(g)

                    if li == 0 and g == 0:
                        w1sb, w2sb = w1sb0, w2sb0
                    else:
                        w1d, w2d = wdram[li]
                        if pl.g1_fp8:
                            p0, pn = b1_0 // 2, b1_n // 2
                            w1sb = sb.tile([128, pn, 2, 128], FP8,
                                           name=f"w1sb_{li}_{g}", tag="w1")
                            hp = (pn + 1) // 2
                            nc.sync.dma_start(out=w1sb[:, 0:hp],
                                              in_=w1d[:, p0:p0 + hp])
                            nc.sync.dma_start(out=w1sb[:, hp:pn],
                                              in_=w1d[:, p0 + hp:p0 + pn])
                        else:
                            w1sb = sb.tile([128, b1_n * 128], ACT_DT,
                                           name=f"w1sb_{li}_{g}", tag="w1")
                            hn = (b1_n + 1) // 2
                            nc.sync.dma_start(
                                out=w1sb[:, 0:hn * 128],
                                in_=w1d[:, b1_0 * 128:(b1_0 + hn) * 128])
                            nc.sync.dma_start(
                                out=w1sb[:, hn * 128:b1_n * 128],
                                in_=w1d[:,
                                        (b1_0 + hn) * 128:(b1_0 + b1_n) * 128])
                        if li == 0 and L1G2_NP8 > 0:
                            w2d8, w2dB = w2d
                            w2sb = (
                                sb.tile([128, 12, L1G2_NP8, 2, 128], FP8,
                                        name=f"w2sb8_{li}_{g}", tag="w28"),
                                sb.tile([128, 12 * (6 - NP8_2) * 128],
                                        ACT_DT, name=f"w2sb_{li}_{g}",
                                        tag="w2"))
                            nc.sync.dma_start(
                                out=w2sb[0][:, :],
                                in_=w2d8[:, 12 * g:12 * (g + 1)])
                            nbg = 12 * (6 - NP8_2)
                            base = g * nbg * 128
                            nc.sync.dma_start(
                                out=w2sb[1][:, 0:nbg * 64],
                                in_=w2dB[:, base:base + nbg * 64])
                            nc.sync.dma_start(
                                out=w2sb[1][:, nbg * 64:nbg * 128],
                                in_=w2dB[:, base + nbg * 64:base + nbg * 128])
                        else:
                            w2sb = sb.tile([128, b2_n * 128], ACT_DT,
                                           name=f"w2sb_{li}_{g}", tag="w2")
                            hn2 = (b2_n + 1) // 2
                            nc.sync.dma_start(
                                out=w2sb[:, 0:hn2 * 128],
                                in_=w2d[:, b2_0 * 128:(b2_0 + hn2) * 128])
                            nc.sync.dma_start(
                                out=w2sb[:, hn2 * 128:b2_n * 128],
                                in_=w2d[:,
                                        (b2_0 + hn2) * 128:(b2_0 + b2_n) * 128])

                    def alloc_mids(l, pl=pl, li=li):
                        """Per-l mid buffers; for L1 the first 2*NP8 tiles
                        live in a separate fp8 buffer."""
                        mts_l = pl.mid_tiles_of_l[l]
                        ent = {}
                        if li == 0 and L1G2_NP8 > 0:
                            n8 = 2 * L1G2_NP8
                            m8 = sb.tile([128, n8, BPC], FP8,
                                         name=f"mid8_{li}_{l}", tag="mid8",
                                         bufs=2)
                            mb = sb.tile([128, len(mts_l) - n8, BPC], ACT_DT,
                                         name=f"mid_{li}_{l}", tag="midb",
                                         bufs=2)
                            for loc, mt in enumerate(mts_l):
                                ent[mt] = (m8, loc) if loc < n8 \
                                    else (mb, loc - n8)
                        else:
                            mb = sb.tile([128, len(mts_l), BPC], ACT_DT,
                                         name=f"mid_{li}_{l}", tag="midb",
                                         bufs=2)
                            for loc, mt in enumerate(mts_l):
                                ent[mt] = (mb, loc)
                        return ent

                    if li == 0 and g == 0:
                        # interleave G1 chains across the two l's so the
                        # earliest-arriving x tiles feed as much work as
                        # possible; G2 for both l's afterwards.
                        mid_of = {}
                        for l in ls:
                            mid_of.update(alloc_mids(l))
                        for mt in g0_mts:
                            midl, loc = mid_of[mt]
                            g1_tile(mt, loc, midl, w1sb, b1_0)
                        for l in ls:
                            for ot in pl.out_tiles_of_l[l]:
                                g2_tile(ot, mid_of, w2sb, b2_0)
                    elif pl.shared_mid:
                        # k-pure mid tiles shared between l's: emit each mid
                        # tile once, then the out tiles whose chains are
                        # complete (interleaved for the 8-tile layout).
                        nmt = pl.lmid.ntiles
                        if nmt == 8:
                            midA = sb.tile([128, 4, BPC], ACT_DT,
                                           name=f"mid_{li}_a", tag="midb",
                                           bufs=2)
                            midB = sb.tile([128, 4, BPC], ACT_DT,
                                           name=f"mid_{li}_b", tag="midb",
                                           bufs=2)
                            mid_of = {mt: ((midA, mt // 2) if mt % 2 == 0
                                           else (midB, mt // 2))
                                      for mt in range(nmt)}
                            order = [("m", 0), ("m", 2), ("m", 4), ("m", 6),
                                     ("m", 1), ("o", 0), ("m", 3), ("o", 1),
                                     ("m", 5), ("o", 2), ("m", 7), ("o", 3),
                                     ("o", 4), ("o", 5), ("o", 6), ("o", 7)]
                        else:
                            midA = sb.tile([128, nmt, BPC], ACT_DT,
                                           name=f"mid_{li}_a", tag="midb",
                                           bufs=2)
                            mid_of = {mt: (midA, mt) for mt in range(nmt)}
                            order = ([("m", mt) for mt in range(nmt)] +
                                     [("o", ot)
                                      for ot in range(pl.lout.ntiles)])
                        for kind, idx in order:
                            if kind == "m":
                                midl, loc = mid_of[idx]
                                g1_tile(idx, loc, midl, w1sb, b1_0)
                            else:
                                g2_tile(idx, mid_of, w2sb, b2_0)
                    else:
                        # per-l pipeline with one-block lookahead
                        mid_of = {}
                        pend = None
                        for l in ls:
                            mid_of.update(alloc_mids(l))
                            for mt in pl.mid_tiles_of_l[l]:
                                midl, loc = mid_of[mt]
                                g1_tile(mt, loc, midl, w1sb, b1_0)
                            if pend is not None:
                                for ot in pl.out_tiles_of_l[pend]:
                                    g2_tile(ot, mid_of, w2sb, b2_0)
                            pend = l
                        for ot in pl.out_tiles_of_l[pend]:
                            g2_tile(ot, mid_of, w2sb, b2_0)

                hin = hnext

            # ---- tail: dense layers + form-B logits + log_softmax,
            # pipelined over the two 512-column batch chunks so the first
            # chunk's serial DVE softmax chain overlaps the second chunk's
            # PE work ----
            from concourse.bass import broadcast_tensor_aps
            wsb_d = {}
            for li in range(NMONARCH, NLAYERS - 1):
                pl = plans[li]
                wsb_d[li] = sb.tile([128, 128 * len(pl.blocks)], ACT_DT,
                                    name=f"wd_{li}", tag="wd", bufs=3)
                nc.sync.dma_start(out=wsb_d[li][:, :], in_=wdram[li][:, :])
            w7sb = sb.tile([128, NOUT], ACT_DT, name="w7", tag="wd", bufs=3)
            nc.sync.dma_start(out=w7sb[:, :], in_=wdram[NLAYERS - 1][:, :])

            hs = {}
            hcur = hin
            for li in range(NMONARCH, NLAYERS - 1):
                pl = plans[li]
                hnext = sb.tile([128, pl.lout.ntiles, BPC], ACT_DT,
                                name=f"h_{li + 1}",
                                tag="hB" if li % 2 == 0 else "hA")
                hs[li] = (hcur, hnext)
                hcur = hnext
            h6 = hcur
            pl7 = plans[NLAYERS - 1]
            ln7 = pl7.in_valid

            nch = BPC // 128
            hc = nch // 2
            logit = sb.tile([128, nch, NOUT], F32, name="logit", tag="logit")
            esb = sb.tile([128, nch, NOUT], F32, name="esb", tag="esb")
            esum = sb.tile([128, nch], F32, name="esum", tag="esum")
            y0 = sb.tile([128, nch], F32, name="y0", tag="y0")
            ey = sb.tile([128, nch], F32, name="ey", tag="ey")
            r = sb.tile([128, nch], F32, name="r", tag="r")
            d = sb.tile([128, nch], F32, name="d", tag="d")
            s1 = sb.tile([128, nch], F32, name="s1", tag="s1")
            q = sb.tile([128, nch], F32, name="q", tag="q")
            lns = sb.tile([128, nch], F32, name="lns", tag="lns")
            osb = sb.tile([128, nch, NOUT], F32, name="osb", tag="osb")

            def dense_cs(li, cs):
                pl = plans[li]
                hi, hn = hs[li]
                c0 = cs * 512
                for ot in range(pl.lout.ntiles):
                    V = int(pl.lout.valid[ot])
                    po = ps.tile([128, 512], F32, name=f"po_{li}",
                                 tag="pout", bufs=3)
                    for j, it in enumerate(range(pl.lin.ntiles)):
                        ln = int(pl.lin.valid[it])
                        b = pl.block_of[(ot, it)]
                        nc.tensor.matmul(po[0:V, :],
                                         wsb_d[li][0:ln, b * 128:b * 128 + V],
                                         hi[0:ln, it, c0:c0 + 512],
                                         start=(j == 0),
                                         stop=(j == pl.lin.ntiles - 1))
                    evict(hn[0:V, ot, c0:c0 + 512], po[0:V, :], relu=True)

            def l7_cs(cs):
                # form-B logits for this chunk's four 128-row sub-chunks
                for ch in range(cs * hc, (cs + 1) * hc):
                    po = ps.tile([128, NOUT], F32, name="po7", tag="pout",
                                 bufs=3)
                    nc.tensor.matmul(po[:, :],
                                     h6[0:ln7, 0, ch * 128:(ch + 1) * 128],
                                     w7sb[0:ln7, :],
                                     start=True, stop=True)
                    evict(logit[:, ch, :], po[:, :], relu=False)

            def softmax_cs(cs):
                # S = sum(exp(t)); ln S via exponent-bits estimate + one
                # Newton step (only Exp needed: a single act-table set).
                cc = slice(cs * hc, (cs + 1) * hc)
                nc.scalar.activation(esb[:, cc, :], logit[:, cc, :],
                                     mybir.ActivationFunctionType.Exp)
                nc.vector.tensor_reduce(esum[:, cc], esb[:, cc, :],
                                        axis=mybir.AxisListType.X,
                                        op=mybir.AluOpType.add)
                nc.vector.tensor_scalar(y0[:, cc], esum.bitcast(I32)[:, cc],
                                        EXP_BITS_BIAS, LN2_OVER_2P23,
                                        op0=mybir.AluOpType.subtract,
                                        op1=mybir.AluOpType.mult)
                nc.scalar.activation(ey[:, cc], y0[:, cc],
                                     mybir.ActivationFunctionType.Exp,
                                     scale=-1.0)
                nc.vector.tensor_tensor(r[:, cc], esum[:, cc], ey[:, cc],
                                        op=mybir.AluOpType.mult)
                nc.vector.tensor_scalar_add(d[:, cc], r[:, cc], -1.0)
                nc.vector.tensor_tensor(s1[:, cc], d[:, cc], y0[:, cc],
                                        op=mybir.AluOpType.add)
                nc.vector.scalar_tensor_tensor(q[:, cc], d[:, cc], -0.5,
                                               d[:, cc],
                                               op0=mybir.AluOpType.mult,
                                               op1=mybir.AluOpType.mult)
                nc.vector.tensor_tensor(lns[:, cc], s1[:, cc], q[:, cc],
                                        op=mybir.AluOpType.add)
                lg_ap, ln_ap = broadcast_tensor_aps(
                    logit[:, cc, :],
                    lns[:, cc].rearrange("p (c u) -> p c u", u=1))
                nc.vector.tensor_tensor(osb[:, cc, :], lg_ap, ln_ap,
                                        op=mybir.AluOpType.subtract)
                nc.sync.dma_start(out=y[:, cc, :], in_=osb[:, cc, :])

            # dense layers run both chunks back-to-back; the chunk-0 softmax
            # DVE chain then overlaps chunk 1's form-B PE work.
            for li in range(NMONARCH, NLAYERS - 1):
                dense_cs(li, 0)
                dense_cs(li, 1)
            l7_cs(0)
            softmax_cs(0)
            l7_cs(1)
            softmax_cs(1)
    nc.finalize()
    return nc


# ------------------------------------------------------------------ entry point
def _prep_inputs(inputs, plans):
    np_dt = mybir.dt.np(ACT_DT)
    np_dt8 = mybir.dt.np(FP8)
    x_np_dt = np_dt8 if L1G1_FP8 else np_dt
    x = np.ascontiguousarray(np.asarray(inputs["x"], dtype=np.float32))
    shared = {}
    for i, pl in enumerate(plans):
        w1 = np.asarray(inputs[f"w1_{i + 1}"], dtype=np.float32)
        w2 = np.asarray(inputs[f"w2_{i + 1}"], dtype=np.float32)
        if isinstance(pl, LayerPlan):
            W1m, W2m = pl.build_weights(w1, w2)
            w1_dt = np_dt8 if pl.g1_fp8 else np_dt
            shared[f"w1c_{i}"] = np.ascontiguousarray(W1m.astype(w1_dt))
            if isinstance(W2m, tuple):
                W2m8, W2mB = W2m
                shared[f"w2c8_{i}"] = np.ascontiguousarray(
                    W2m8.astype(np_dt8))
                shared[f"w2c_{i}"] = np.ascontiguousarray(W2mB.astype(np_dt))
            else:
                shared[f"w2c_{i}"] = np.ascontiguousarray(W2m.astype(np_dt))
        elif isinstance(pl, DensePlan):
            Wm = pl.build_weights(w1, w2)
            shared[f"wdc_{i}"] = np.ascontiguousarray(Wm.astype(np_dt))
        else:
            W = pl.build_weights(w1, w2)
            shared[f"w7c_{i}"] = np.ascontiguousarray(W.astype(np_dt))
    in_maps = []
    for c in range(NCORES):
        m = dict(shared)
        xc = x[c * BPC:(c + 1) * BPC].T.astype(x_np_dt)    # [3072, 1024]
        m["xT"] = np.ascontiguousarray(
            xc.reshape(plans[0].lin.ntiles, 128, BPC).transpose(1, 0, 2))
        in_maps.append(m)
    return in_maps


def _run(inputs, trace=False, **spmd_kwargs):
    plans = build_plans()
    in_maps = _prep_inputs(inputs, plans)
    nc = build_program(plans)
    res = run_bass_kernel_spmd(nc, in_maps, core_ids=list(range(NCORES)),
                               trace=trace, **spmd_kwargs)
    nch = BPC // 128
    out = np.concatenate(
        [np.asarray(r["y"]).reshape(128, nch, NOUT)
         .transpose(1, 0, 2).reshape(BPC, NOUT)
         for r in res.results], axis=0)
    return out.astype(np.float32), res


def kernel(**inputs):
    out, _ = _run(inputs, trace=False)
    return out

